# revision 11
# baseline (speedup 1.0000x reference)
"""Trainium2 Bass kernel for DiagonalLinear.

The reference masks W to its diagonal (zeroing entries with |w| <= 1e-4)
and computes x @ masked_W.T, which is exactly an elementwise scale of
x's columns by the thresholded diagonal of W.

Distribution (8 NeuronCores): data-parallel — x is sharded along the
token axis (1024 tokens per core); per the sharding hint, only the
(thresholded) diagonal of W — the sole part of W the op reads — is
replicated to every core. No inter-core communication.

The op is purely memory-bound. bf16 streaming (x and out quantized on
the host; the roundings stay under 1.2%, inside the 2e-2 tolerance)
puts per-core traffic at 8 MiB in + 8 MiB out. The combined load+store
stream saturates the 16 SBUF AXI ports (~425 GB/s measured), so the
whole game is keeping both DMA queues fed at that rate for the entire
window. Design, driven by trace measurements:

1. The diagonal arrives from HBM already replicated across the 128
   partitions ([128, 4096] bf16, 1 MiB, host-prepared). The on-device
   alternative (row load + K=1 matmul broadcast + PSUM->SBUF casts)
   costs ~6 us of serial latency before the first multiply can run.
   The extra 1 MiB costs only ~2.4 us of stream time and frees the
   DVE to do nothing but multiplies. Tensor/PSUM unused.

2. DMA completion semaphores land ~1.5-3 us after the last byte (HBM
   receipt round trip), so every multiply is load-SEM-paced, not
   load-byte-paced. The head is therefore row-split into small
   CONTIGUOUS DMAs (column-split loads measured only ~320 GB/s — 4 KB
   strided HBM reads lose page efficiency): d_rep and the first x tile
   each load as two [64, 4096] halves. The first [64, 2048] multiply
   fires at ~5 us and the store stream is flowing by ~7 us, so load
   and store packets interleave for essentially the whole window.

3. A ring's FIRST DMA pays a ~4.5 us (qAct) / ~1.5 us (qSP) cold
   start before bytes move (measured). The scalar engine issues a
   no-wait dummy write (uninitialized scratch -> DRAM scratch) at t=0
   so the qAct ring is warm before the first output store needs it.

4. Tile rows [64,64,128,128,120,120,128,128,128,16]: the 120-row
   tiles (15-engine, port-crossed descriptor layout — full rate only
   when load and store packets interleave, measured ~215-250 GB/s
   single-queue) sit mid-stream where stores are active. Engine 15
   (the measured slower SDMA engine) gets 49/65 of the per-engine
   line count, matching its speed, so it is never the straggler. The
   tiny [16] tile goes last: the final load->mul->store chain is
   short and hides under the store-backlog drain.

5. Multiplies and stores run per column-half (a DVE tensor op's time
   scales with free-dim length, so halves cost no extra throughput;
   column-half STORES measured full rate — HBM writes tolerate the
   4 KB stride) — the store FIFO is fed at fine granularity and each
   unit's store issues right after its multiply. The last three store
   units ride the sync ring once its loads have drained, so the tail
   backlog drains through both rings.

Per-core device program — raw Bass (no Tile scheduler) with hand-placed
semaphores; the kernel ends on store-completion waits, not an
all-engine barrier.

Engine plan (single Block):
  sync   : 2 d_rep half-loads + 11 x-tile loads on the qSP ring, a
           warm-up write, then the last 3 store units
  scalar : no-wait qAct warm-up write at t=0, then store units 0..16
           (each gated on its multiply)
  vector : the 20 column-half multiplies, each gated on its tile load
  tensor : idle
"""

import numpy as np

TOKENS = 8192
N = 4096
N_CORES = 8
T_SHARD = TOKENS // N_CORES  # 1024
TILE_P = [64, 64, 128, 128, 120, 120, 128, 128, 128, 16]
P0 = 128
THRESHOLD = 1e-4
N_SYNC_STORES = 3            # tail store units issued on the sync ring

_CACHED_NC = None


def _build_nc(tile_p=None, n_sync_stores=N_SYNC_STORES):
    from contextlib import ExitStack

    from concourse import bass, mybir

    bf16 = mybir.dt.bfloat16
    tile_p = list(TILE_P) if tile_p is None else list(tile_p)
    assert sum(tile_p) == T_SHARD
    nc = bass.Bass()
    x_in = nc.declare_dram_parameter("x", [T_SHARD, N], bf16, isOutput=False)
    d_in = nc.declare_dram_parameter("drep", [P0, N], bf16, isOutput=False)
    out = nc.declare_dram_parameter("out", [T_SHARD, N], bf16, isOutput=True)
    warm = nc.dram_tensor("warm", [2, N], bf16)  # warm-up write target

    x_ap = x_in[:]
    o_ap = out[:]
    offs = np.cumsum([0] + tile_p)
    x_v = [x_ap[offs[i] : offs[i + 1]] for i in range(len(tile_p))]
    o_v = [o_ap[offs[i] : offs[i + 1]] for i in range(len(tile_p))]

    n_tiles = len(tile_p)
    H = N // 2
    h0, h1 = slice(0, H), slice(H, N)
    # (tile, col_slice) units in mul/store order: two column-halves per tile
    units = [(t, cs) for t in range(n_tiles) for cs in (h0, h1)]
    n_mul = len(units)                    # 20
    n_scalar_units = n_mul - n_sync_stores

    with ExitStack() as ctx:
        s_dt = ctx.enter_context(nc.semaphore("s_dt"))  # d_rep rows 0-63
        s_db = ctx.enter_context(nc.semaphore("s_db"))  # d_rep rows 64-127
        s_ld = [
            ctx.enter_context(nc.semaphore(f"s_ld{i}")) for i in range(n_tiles)
        ]
        s_mul = ctx.enter_context(nc.semaphore("s_mul"))
        # store-completion counts can exceed a sem's range in one counter
        # (17 units x 16 = 272), so the scalar stores split across two
        s_st_a = ctx.enter_context(nc.semaphore("s_st_a"))
        s_st_b = ctx.enter_context(nc.semaphore("s_st_b"))
        s_st2 = ctx.enter_context(nc.semaphore("s_st2"))
        s_warm = ctx.enter_context(nc.semaphore("s_warm"))

        db = ctx.enter_context(nc.sbuf_tensor("db", [P0, N], bf16))
        # dedicated never-written scratch: the t=0 warm-ups read it
        # (contents irrelevant; target is DRAM scratch)
        wsrc = ctx.enter_context(nc.sbuf_tensor("wsrc", [1, N], bf16))
        xts = [
            ctx.enter_context(nc.sbuf_tensor(f"xt{i}", [p, N], bf16))
            for i, p in enumerate(tile_p)
        ]

        HP = P0 // 2  # 64
        n_sta = n_scalar_units // 2

        with nc.Block() as block:

            @block.sync
            def _(sync):
                # row-split head: small contiguous DMAs so the first
                # multiplies' sems land early despite the receipt lag
                sync.dma_start(out=db[:HP], in_=d_in[:HP]).then_inc(s_dt, 16)
                sync.dma_start(out=xts[0][:], in_=x_v[0]).then_inc(s_ld[0], 16)
                sync.dma_start(out=db[HP:], in_=d_in[HP:]).then_inc(s_db, 16)
                for i in range(1, n_tiles):
                    sync.dma_start(out=xts[i][:], in_=x_v[i]).then_inc(
                        s_ld[i], 16
                    )
                sync.dma_start(out=warm[0, None, :], in_=wsrc[:]).then_inc(
                    s_warm, 16
                )
                # tail stores ride the sync ring: it is idle once the
                # loads drain, so the store backlog drains on both rings
                for k in range(n_scalar_units, n_mul):
                    t, cs = units[k]
                    sync.wait_ge(s_mul, k + 1)
                    sync.dma_start(out=o_v[t][:, cs], in_=xts[t][:, cs]).then_inc(
                        s_st2, 16
                    )
                sync.wait_ge(s_st2, 16 * n_sync_stores)
                sync.wait_ge(s_warm, 32)

            @block.scalar
            def _(scalar):
                # no-wait warm-up: the first DMA on the qAct ring pays a
                # ~4.5 us cold start, so burn it at t=0 on a dummy write
                # instead of on the first output store
                scalar.dma_start(out=warm[1, None, :], in_=wsrc[:]).then_inc(
                    s_warm, 16
                )
                for k in range(n_scalar_units):
                    t, cs = units[k]
                    scalar.wait_ge(s_mul, k + 1)
                    sem = s_st_a if k < n_sta else s_st_b
                    scalar.dma_start(
                        out=o_v[t][:, cs], in_=xts[t][:, cs]
                    ).then_inc(sem, 16)
                scalar.wait_ge(s_st_a, 16 * n_sta)
                scalar.wait_ge(s_st_b, 16 * (n_scalar_units - n_sta))
                scalar.wait_ge(s_warm, 32)

            @block.vector
            def _(vector):
                db_waited = False
                for k, (t, cs) in enumerate(units):
                    p = tile_p[t]
                    if cs is h0:
                        vector.wait_ge(s_ld[t], 16)
                    if k == 0:
                        vector.wait_ge(s_dt, 16)
                    elif p > HP and not db_waited:
                        # first unit touching d_rep rows 64-127
                        vector.wait_ge(s_db, 16)
                        db_waited = True
                    vector.tensor_mul(
                        out=xts[t][:, cs], in0=xts[t][:, cs], in1=db[:p, cs]
                    ).then_inc(s_mul, 1)

    nc.finalize()
    return nc


def _get_nc():
    global _CACHED_NC
    if _CACHED_NC is None:
        _CACHED_NC = _build_nc()
    return _CACHED_NC


def _shard_inputs(x, W):
    import ml_dtypes

    bf16 = ml_dtypes.bfloat16
    x = np.ascontiguousarray(np.asarray(x, dtype=np.float32)).astype(bf16)
    W = np.asarray(W, dtype=np.float32)
    d = np.ascontiguousarray(np.diagonal(W))
    d = np.where(np.abs(d) > THRESHOLD, d, np.float32(0.0)).astype(bf16)
    drep = np.ascontiguousarray(np.broadcast_to(d[None, :], (P0, N)))
    assert x.shape == (TOKENS, N) and drep.shape == (P0, N)
    return [
        {"x": x[c * T_SHARD : (c + 1) * T_SHARD], "drep": drep}
        for c in range(N_CORES)
    ]


def _run(x, W, **spmd_kwargs):
    from concourse.bass_utils import run_bass_kernel_spmd

    nc = _get_nc()
    in_maps = _shard_inputs(x, W)
    res = run_bass_kernel_spmd(nc, in_maps, list(range(N_CORES)), **spmd_kwargs)
    out = np.concatenate(
        [res.results[c]["out"] for c in range(N_CORES)], axis=0
    ).astype(np.float32)
    return out, res


def kernel(x, W):
    out, _ = _run(x, W)
    return out


# revision 12
# speedup vs baseline: 1.1097x; 1.1097x over previous
"""Trainium2 Bass kernel for DiagonalLinear.

The reference masks W to its diagonal (zeroing entries with |w| <= 1e-4)
and computes x @ masked_W.T, which is exactly an elementwise scale of
x's columns by the thresholded diagonal of W.

Distribution (8 NeuronCores): data-parallel — x is sharded along the
token axis (1024 tokens per core); per the sharding hint, only the
(thresholded) diagonal of W — 4096 floats, the sole part of W the op
reads — is replicated to every core. No inter-core communication.

The op is purely memory-bound: per-core traffic is 8 MiB in + 8 MiB
out of bf16 (host-quantized; the roundings stay under 1.2%, inside
the 2e-2 tolerance), and the combined load+store stream saturates the
16 SBUF AXI ports at ~425 GB/s. Trace-measured facts this schedule is
built on:

1. Only [128]-row full-width DMAs sustain ~425 GB/s on a single
   queue. Other shapes degrade when one queue runs alone: [120]-row
   ~215-250 (port-crossed 15-engine layout), [64]-row ~210-270,
   column-split halves ~300-340 (4 KB strided HBM reads). So ALL x
   tiles load as [128, 4096] except one [120] mid-stream and one [8]
   at the end (engine-15 relief, see 4).

2. Loads structurally out-compete stores for DMA slots (the store
   FIFO runs shallow because each store is issued only after its
   multiply's semaphore). That is harmless — loads just run ahead at
   full rate and the store backlog drains in a store-only tail at
   ~410 GB/s — PROVIDED every load shape is full-rate solo (point 1).
   The window is then simply total-bytes / ~420 GB/s.

3. The diagonal path is latency- not bandwidth-critical: an 8 KB
   d-row load heads the sync FIFO, TensorE replicates it across
   partitions with 8 exact K=1 matmuls (ones[1,128]^T @ d_row[1,512]
   -> PSUM banks), ACT casts PSUM->SBUF bf16 in two column halves,
   and the first two multiplies read PSUM directly so they only gate
   on the matmuls. This costs no HBM traffic (vs 1 MiB = +2.4 us of
   window for a host-replicated diagonal, which measured net-slower).

4. Engine 15 is the slowest SDMA engine (measured ~7-18%). With
   all-[128] tiles it would straggle every semaphore. The single
   [120,4096] tile (engine 15 idle) plus the [8,4096] tile (engines
   0-7 only) shed exactly enough bytes that engine 15 finishes with
   the pack. The [120] tile sits mid-stream where stores interleave
   (its port-crossed layout only hits full rate in mixed traffic);
   the [8] tile is last so the final load->mul->store chain is tiny
   and hides under the store-backlog drain.

5. A ring's FIRST DMA pays a ~4.5 us (qAct) / ~1.5 us (qSP) cold
   start before bytes move. The scalar engine issues a no-wait dummy
   write (uninitialized scratch -> DRAM scratch) at t=0 so the qAct
   ring is warm before the first output store needs it.

6. Multiplies and stores run per column-half (512 KB units; DVE op
   time scales with free-dim length so halves cost no throughput, and
   column-half STORES measured full rate — HBM writes tolerate the
   stride). In the tail the DVE (427 GB/s effective) must outpace the
   store drain (~410 GB/s), which it does — back-to-back 1.2 us
   multiplies with no inter-op gap. The last three store units ride
   the sync ring once its loads have drained.

Per-core device program — raw Bass (no Tile scheduler) with hand-placed
semaphores; the kernel ends on store-completion waits, not an
all-engine barrier.

Engine plan (single Block):
  sync   : d-row load then 9 x-tile loads on the qSP ring, a warm-up
           write, then the last 3 store units
  tensor : the 8 diagonal-broadcast matmuls
  scalar : no-wait qAct warm-up at t=0, 2 PSUM->SBUF casts, then
           store units 0..14 (each gated on its multiply)
  vector : the 18 column-half multiplies (first two PSUM-direct),
           each gated on its tile load
"""

import numpy as np

TOKENS = 8192
N = 4096
N_CORES = 8
T_SHARD = TOKENS // N_CORES  # 1024
TILE_P = [128, 128, 128, 128, 120, 128, 128, 128, 8]
P0 = 128
MM_N = 512                   # PSUM bank width (fp32)
THRESHOLD = 1e-4
N_SYNC_STORES = 3            # tail store units issued on the sync ring

_CACHED_NC = None


def _build_nc(tile_p=None, n_sync_stores=N_SYNC_STORES):
    from contextlib import ExitStack

    from concourse import bass, mybir

    bf16 = mybir.dt.bfloat16
    f32 = mybir.dt.float32
    tile_p = list(TILE_P) if tile_p is None else list(tile_p)
    assert sum(tile_p) == T_SHARD
    nc = bass.Bass()
    x_in = nc.declare_dram_parameter("x", [T_SHARD, N], bf16, isOutput=False)
    d_in = nc.declare_dram_parameter("d", [N], bf16, isOutput=False)
    out = nc.declare_dram_parameter("out", [T_SHARD, N], bf16, isOutput=True)
    warm = nc.dram_tensor("warm", [2, N], bf16)  # warm-up write target

    x_ap = x_in[:]
    o_ap = out[:]
    offs = np.cumsum([0] + tile_p)
    x_v = [x_ap[offs[i] : offs[i + 1]] for i in range(len(tile_p))]
    o_v = [o_ap[offs[i] : offs[i + 1]] for i in range(len(tile_p))]

    n_tiles = len(tile_p)
    H = N // 2
    h0, h1 = slice(0, H), slice(H, N)
    # (tile, col_slice) units in mul/store order: two column-halves per tile
    units = [(t, cs) for t in range(n_tiles) for cs in (h0, h1)]
    n_mul = len(units)                    # 18
    n_scalar_units = n_mul - n_sync_stores
    n_sta = n_scalar_units // 2

    with ExitStack() as ctx:
        s_ld = [
            ctx.enter_context(nc.semaphore(f"s_ld{i}")) for i in range(n_tiles)
        ]
        s_row = ctx.enter_context(nc.semaphore("s_row"))
        s_ones = ctx.enter_context(nc.semaphore("s_ones"))
        s_mm = ctx.enter_context(nc.semaphore("s_mm"))
        s_cp = ctx.enter_context(nc.semaphore("s_cp"))
        s_mul = ctx.enter_context(nc.semaphore("s_mul"))
        s_st_a = ctx.enter_context(nc.semaphore("s_st_a"))
        s_st_b = ctx.enter_context(nc.semaphore("s_st_b"))
        s_st2 = ctx.enter_context(nc.semaphore("s_st2"))
        s_warm = ctx.enter_context(nc.semaphore("s_warm"))

        row = ctx.enter_context(nc.sbuf_tensor("row", [1, N], bf16))
        ones = ctx.enter_context(nc.sbuf_tensor("ones", [1, P0], bf16))
        db = ctx.enter_context(nc.sbuf_tensor("db", [P0, N], bf16))
        # dedicated never-written scratch: the t=0 warm-up reads it
        wsrc = ctx.enter_context(nc.sbuf_tensor("wsrc", [1, N], bf16))
        xts = [
            ctx.enter_context(nc.sbuf_tensor(f"xt{i}", [p, N], bf16))
            for i, p in enumerate(tile_p)
        ]
        acc = ctx.enter_context(nc.psum_tensor("acc", [P0, N], f32))

        with nc.Block() as block:

            @block.sync
            def _(sync):
                # d-row load heads the load FIFO: its 16 descriptors
                # complete in the first packet round (~1 us)
                sync.dma_start(out=row[:], in_=d_in[None, :]).then_inc(s_row, 16)
                for i in range(n_tiles):
                    sync.dma_start(out=xts[i][:], in_=x_v[i]).then_inc(s_ld[i], 16)
                sync.wait_ge(s_row, 16)
                sync.dma_start(out=warm[0, None, :], in_=wsrc[:]).then_inc(
                    s_warm, 16
                )
                # tail stores ride the sync ring: it is idle once the
                # loads drain, so the store backlog drains on both rings
                for k in range(n_scalar_units, n_mul):
                    t, cs = units[k]
                    sync.wait_ge(s_mul, k + 1)
                    sync.dma_start(out=o_v[t][:, cs], in_=xts[t][:, cs]).then_inc(
                        s_st2, 16
                    )
                sync.wait_ge(s_st2, 16 * n_sync_stores)
                sync.wait_ge(s_warm, 32)

            @block.tensor
            def _(tensor):
                tensor.wait_ge(s_ones, 1)
                tensor.wait_ge(s_row, 16)
                for j in range(N // MM_N):
                    tensor.matmul(
                        acc[:, j * MM_N : (j + 1) * MM_N],
                        ones[:],
                        row[:, j * MM_N : (j + 1) * MM_N],
                        start=True,
                        stop=True,
                    ).then_inc(s_mm, 1)

            @block.scalar
            def _(scalar):
                # no-wait warm-up: the first DMA on the qAct ring pays a
                # ~4.5 us cold start, so burn it at t=0 on a dummy write
                scalar.dma_start(out=warm[1, None, :], in_=wsrc[:]).then_inc(
                    s_warm, 16
                )
                # PSUM -> SBUF bf16 broadcast casts (exact: f32 holds the
                # bf16 values); the first two multiplies read PSUM
                # directly so they don't wait on these
                for c in range(2):
                    scalar.wait_ge(s_mm, (c + 1) * 4)
                    scalar.copy(
                        out=db[:, c * H : (c + 1) * H],
                        in_=acc[:, c * H : (c + 1) * H],
                    ).then_inc(s_cp, 1)
                for k in range(n_scalar_units):
                    t, cs = units[k]
                    scalar.wait_ge(s_mul, k + 1)
                    sem = s_st_a if k < n_sta else s_st_b
                    scalar.dma_start(
                        out=o_v[t][:, cs], in_=xts[t][:, cs]
                    ).then_inc(sem, 16)
                scalar.wait_ge(s_st_a, 16 * n_sta)
                scalar.wait_ge(s_st_b, 16 * (n_scalar_units - n_sta))
                scalar.wait_ge(s_warm, 32)

            @block.vector
            def _(vector):
                vector.memset(ones[:], 1.0).then_inc(s_ones, 1)
                for k, (t, cs) in enumerate(units):
                    p = tile_p[t]
                    if cs is h0:
                        vector.wait_ge(s_ld[t], 16)
                    if k <= 1:
                        # first two multiplies read the replicated
                        # diagonal straight from PSUM, gated only on the
                        # matmuls
                        vector.wait_ge(s_mm, 4 * (k + 1))
                        src_d = acc[:p, cs]
                    else:
                        if k == 2:
                            vector.wait_ge(s_cp, 1)
                        elif k == 3:
                            vector.wait_ge(s_cp, 2)
                        src_d = db[:p, cs]
                    vector.tensor_mul(
                        out=xts[t][:, cs], in0=xts[t][:, cs], in1=src_d
                    ).then_inc(s_mul, 1)

    nc.finalize()
    return nc


def _get_nc():
    global _CACHED_NC
    if _CACHED_NC is None:
        _CACHED_NC = _build_nc()
    return _CACHED_NC


def _shard_inputs(x, W):
    import ml_dtypes

    bf16 = ml_dtypes.bfloat16
    x = np.ascontiguousarray(np.asarray(x, dtype=np.float32)).astype(bf16)
    W = np.asarray(W, dtype=np.float32)
    d = np.ascontiguousarray(np.diagonal(W))
    d = np.where(np.abs(d) > THRESHOLD, d, np.float32(0.0)).astype(bf16)
    assert x.shape == (TOKENS, N) and d.shape == (N,)
    return [
        {"x": x[c * T_SHARD : (c + 1) * T_SHARD], "d": d} for c in range(N_CORES)
    ]


def _run(x, W, **spmd_kwargs):
    from concourse.bass_utils import run_bass_kernel_spmd

    nc = _get_nc()
    in_maps = _shard_inputs(x, W)
    res = run_bass_kernel_spmd(nc, in_maps, list(range(N_CORES)), **spmd_kwargs)
    out = np.concatenate(
        [res.results[c]["out"] for c in range(N_CORES)], axis=0
    ).astype(np.float32)
    return out, res


def kernel(x, W):
    out, _ = _run(x, W)
    return out
